# revision 1
# baseline (speedup 1.0000x reference)
import math

import jax
import jax.numpy as jnp
import numpy as np

# nn_DiplomacyPolicyNet — data-parallel over 8 NeuronCores.
# Batch B=1024 is sharded 8 x 128; params/adj replicated (N=81 is tiny).
H = 4
D = 256
DH = D // H
N = 81
F_IN = 47
VOCAB = 169
NLAYERS = 3
EPS = 1e-5
NCORES = 8
B = 1024
U = 17


def _ln(x, g, b):
    m = x.mean(-1, keepdims=True)
    v = x.var(-1, keepdims=True)
    return (x - m) / jnp.sqrt(v + EPS) * g + b


def _gat_block(x, adj_bias, p):
    Bc, n, d = x.shape
    h = (x @ p['W']).reshape(Bc, n, H, DH)
    s_src = jnp.einsum('bnhd,hd->bnh', h, p['a_src'])
    s_dst = jnp.einsum('bnhd,hd->bnh', h, p['a_dst'])
    e = jax.nn.leaky_relu(s_src[:, :, None, :] + s_dst[:, None, :, :], 0.2)
    e = e + adj_bias[None, :, :, None]
    alpha = jax.nn.softmax(e, axis=2)
    gat = jnp.einsum('bijh,bjhd->bihd', alpha, h).reshape(Bc, n, d)
    x = _ln(x + gat, p['ln1_g'], p['ln1_b'])
    ffn = jax.nn.gelu(x @ p['ffn_w1'] + p['ffn_b1'], approximate=False) @ p['ffn_w2'] + p['ffn_b2']
    x = _ln(x + ffn, p['ln2_g'], p['ln2_b'])
    return x


def _forward(board, adj_bias, unit_indices, power_indices, params):
    x = jax.nn.gelu(board @ params['in_w'] + params['in_b'], approximate=False)
    x = _ln(x, params['in_ln_g'], params['in_ln_b'])
    for i in range(NLAYERS):
        x = _gat_block(x, adj_bias, params['blocks'][i])
    Bc, n, d = x.shape
    power_emb = params['power_embed'][power_indices]
    context = x + power_emb[:, None, :]
    safe = jnp.maximum(unit_indices, 0)
    unit_emb = jnp.take_along_axis(
        context, jnp.broadcast_to(safe[:, :, None], (Bc, U, d)), axis=1)
    Q = unit_emb @ params['q_w'] + params['q_b']
    K = context @ params['k_w'] + params['k_b']
    V = context @ params['v_w'] + params['v_b']
    attn = jax.nn.softmax(jnp.einsum('bud,bnd->bun', Q, K) / math.sqrt(d), axis=-1)
    attended = jnp.einsum('bun,bnd->bud', attn, V)
    unit_repr = _ln(unit_emb + attended, params['attn_ln_g'], params['attn_ln_b'])
    hh = jax.nn.gelu(unit_repr @ params['head_w1'] + params['head_b1'], approximate=False)
    hh = _ln(hh, params['head_ln_g'], params['head_ln_b'])
    return hh @ params['head_w2'] + params['head_b2']


_pforward = jax.pmap(_forward, in_axes=(0, None, 0, 0, None))


def kernel(board, adj, unit_indices, power_indices, params):
    board = np.asarray(board, dtype=np.float32).reshape(NCORES, B // NCORES, N, F_IN)
    adj = np.asarray(adj, dtype=np.float32)
    # softmax(where(adj>0, e, -inf)) == softmax(e + additive_bias): saves a
    # [B,N,N,H]-sized select and keeps masked lanes exactly zero after exp.
    adj_bias = np.where(adj > 0, 0.0, -1e30).astype(np.float32)
    unit_indices = np.asarray(unit_indices).astype(np.int32).reshape(NCORES, B // NCORES, U)
    power_indices = np.asarray(power_indices).astype(np.int32).reshape(NCORES, B // NCORES)
    params = jax.tree_util.tree_map(lambda a: np.asarray(a, dtype=np.float32), params)
    out = _pforward(board, adj_bias, unit_indices, power_indices, params)
    return np.asarray(out, dtype=np.float32).reshape(B, U, VOCAB)
